# revision 32
# baseline (speedup 1.0000x reference)
"""MultiHeadAttention on 8 TRN2 NeuronCores.

Shapes (hardcoded): x [4, 2048, 1024], w_qkv [1024, 3072], b_qkv [3072],
w_o [1024, 1024], b_o [1024]; H=16 heads, head_dim=64, scale 1/8.

Sharding: core c -> batch c//2, head-group c%2 (8 heads each).
Each core computes its 8 heads' attention values and a partial o-proj
([2048, 1024] f32); host sums the two partials per batch, adds b_o and
the constant row bv @ w_o (v-bias folds out of attention since softmax
rows sum to 1).

V2 single-phase schedule (all matmuls bf16): projections and o-proj
matmuls are interleaved into the attention steps so the PE array never
idles; the softmax normalization chain of each (pair, query-block) step
is deferred into the next step so the PE does not stall on the DVE
reciprocal; bc_e/bc_o broadcast matmuls stack into one PSUM bank at
partition bases 0/64.
"""

import os
import sys
import types

sys.path.insert(0, "/opt/trn_rl_repo")

import numpy as np
import ml_dtypes
from contextlib import ExitStack

import concourse.bass as bass  # noqa: F401
import concourse.tile as tile
from concourse import bacc, mybir
from concourse.bass_utils import run_bass_kernel_spmd

BF16 = mybir.dt.bfloat16
F32 = mybir.dt.float32
NBF = ml_dtypes.bfloat16

N_CORES = 8
B, S, D, E = 4, 2048, 1024, 1024
H, HD = 16, 64
NP = 4    # head pairs per core
NQB = 4   # query blocks of 512
NKC = 16  # key/seq chunks of 128
NIC = 8   # input-dim chunks of 128

TRACE = os.environ.get("KERNEL_TRACE", "") == "1"
LAST_EXEC_NS = None

if TRACE:
    _hook = [None]
    _ah = types.ModuleType("antenv.axon_hooks")
    _ah.set_axon_ntff_profile_hook = lambda h: _hook.__setitem__(0, h)
    _ah.get_axon_ntff_profile_hook = lambda: _hook[0]
    sys.modules["antenv.axon_hooks"] = _ah
    import antenv
    antenv.axon_hooks = _ah
    from trn_agent_boot.trn_boot import _ntff_profile_via_ctypes
    _ah.set_axon_ntff_profile_hook(
        _ntff_profile_via_ctypes("/opt/axon/libaxon_pjrt.so"))

_nc_cache = [None]


def _build():
    nc = bacc.Bacc("TRN2", target_bir_lowering=False, debug=False,
                   num_devices=N_CORES)
    xT_ap = nc.dram_tensor("xT", [NIC, 128, S], BF16, kind="ExternalInput").ap()
    wq_ap = nc.dram_tensor("wq", [NIC, 128, 512], BF16, kind="ExternalInput").ap()
    wk_ap = nc.dram_tensor("wk", [NIC, 128, 512], BF16, kind="ExternalInput").ap()
    wv_ap = nc.dram_tensor("wv", [NIC, 128, 512], BF16, kind="ExternalInput").ap()
    wo_ap = nc.dram_tensor("wo", [NP, 128, 1024], BF16, kind="ExternalInput").ap()
    bq_ap = nc.dram_tensor("bq", [128, NP], F32, kind="ExternalInput").ap()
    bk_ap = nc.dram_tensor("bk", [128, NP], F32, kind="ExternalInput").ap()
    out_ap = nc.dram_tensor("out", [NKC, 128, 1024], F32,
                            kind="ExternalOutput").ap()

    with tile.TileContext(nc) as tc:
        with ExitStack() as ctx:
            sb = ctx.enter_context(tc.tile_pool(name="sb", bufs=1))
            xT_sb = sb.tile([128, NIC, S], BF16)
            wq_sb = sb.tile([128, NIC, 512], BF16)
            wk_sb = sb.tile([128, NIC, 512], BF16)
            wv_sb = sb.tile([128, NIC, 512], BF16)
            wo_sb = sb.tile([128, NP, 1024], BF16)
            bq_sb = sb.tile([128, NP], F32)
            bk_sb = sb.tile([128, NP], F32)
            v_aug = sb.tile([128, NKC, 8, 65], BF16)
            ones_col = sb.tile([128, 64], BF16)
            rb_f32 = sb.tile([128, 1024], F32)
            rb_b16 = sb.tile([128, 1024], BF16)
            qT = [sb.tile([128, S], BF16, name=f"qT{p}") for p in range(NP)]
            kT = [sb.tile([128, S], BF16, name=f"kT{p}") for p in range(NP)]
            valsT = [sb.tile([128, S], BF16, name=f"valsT{p}")
                     for p in range(NP)]

            for ic in range(NIC):
                nc.sync.dma_start(out=xT_sb[:, ic, :], in_=xT_ap[ic])
                nc.sync.dma_start(out=wv_sb[:, ic, :], in_=wv_ap[ic])
            for ic in range(NIC):
                nc.sync.dma_start(out=wk_sb[:, ic, :], in_=wk_ap[ic])
                nc.sync.dma_start(out=wq_sb[:, ic, :], in_=wq_ap[ic])
            nc.sync.dma_start(out=bq_sb[:], in_=bq_ap[:])
            nc.sync.dma_start(out=bk_sb[:], in_=bk_ap[:])
            for p in range(NP):
                nc.sync.dma_start(out=wo_sb[:, p, :], in_=wo_ap[p])
            nc.gpsimd.memset(v_aug[:], 1.0)
            nc.gpsimd.memset(ones_col[:], 1.0)
            nc.gpsimd.memset(rb_f32[:], 1.0)
            nc.gpsimd.memset(rb_b16[:], 1.0)

            qkps = ctx.enter_context(
                tc.tile_pool(name="qkps", bufs=2, space="PSUM"))
            avps = ctx.enter_context(
                tc.tile_pool(name="avps", bufs=1, space="PSUM"))
            accs = ctx.enter_context(
                tc.tile_pool(name="accs", bufs=2, space="PSUM"))
            eps = ctx.enter_context(tc.tile_pool(name="eps", bufs=10))
            rbs = ctx.enter_context(tc.tile_pool(name="rbs", bufs=2))
            ost = ctx.enter_context(tc.tile_pool(name="ost", bufs=2))

            def vproj_group(kc):
                acc = accs.tile([128, 512], F32, name="acc")
                for ic in range(NIC):
                    nc.tensor.matmul(
                        acc[:], xT_sb[:, ic, kc * 128:(kc + 1) * 128],
                        wv_sb[:, ic, :],
                        start=(ic == 0), stop=(ic == NIC - 1))
                nc.vector.tensor_copy(v_aug[:, kc, :, 0:64], acc[:])

            def proj_group(which, p, qb):
                qcols = slice(qb * 512, (qb + 1) * 512)
                pcols = slice(p * 128, (p + 1) * 128)
                w_sb, b_sb, dst = ((wq_sb, bq_sb, qT) if which == "q"
                                   else (wk_sb, bk_sb, kT))
                acc = accs.tile([128, 512], F32, name="acc")
                for ic in range(NIC):
                    nc.tensor.matmul(
                        acc[:], w_sb[:, ic, pcols], xT_sb[:, ic, qcols],
                        start=(ic == 0), stop=(ic == NIC - 1))
                nc.vector.tensor_scalar_add(
                    dst[p][:, qcols], acc[:], b_sb[:, p:p + 1])

            def oproj_sc(sc):
                scols = slice(sc * 128, (sc + 1) * 128)
                stage = ost.tile([128, 1024], F32, name="ostage")
                for half in range(2):
                    hcols = slice(half * 512, (half + 1) * 512)
                    og = accs.tile([128, 512], F32, name="acc")
                    for p in range(NP):
                        nc.tensor.matmul(
                            og[:], valsT[p][:, scols], wo_sb[:, p, hcols],
                            start=(p == 0), stop=(p == NP - 1))
                    nc.vector.tensor_copy(stage[:, hcols], og[:])
                nc.sync.dma_start(out=out_ap[sc], in_=stage[:])

            pend_norm = [None]

            def make_norm(p, qb, av, recip):
                qcols = slice(qb * 512, (qb + 1) * 512)

                def norm():
                    bc = accs.tile([128, 512], F32, name="acc")
                    nc.tensor.matmul(
                        bc[0:64, :], ones_col[64:65, :],
                        recip[64:65, 0:512], start=True, stop=True)
                    nc.tensor.matmul(
                        bc[64:128, :], ones_col[64:65, :],
                        recip[64:65, 512:1024], start=True, stop=True)
                    bc_sb = rbs.tile([128, 1024], BF16, name="bcsb")
                    nc.vector.tensor_copy(bc_sb[0:64, 0:512], bc[0:64, :])
                    nc.vector.tensor_copy(bc_sb[0:64, 512:1024],
                                          bc[64:128, :])
                    nc.vector.tensor_mul(
                        valsT[p][0:64, qcols], av[0:64, 0:512],
                        bc_sb[0:64, 0:512])
                    nc.vector.tensor_mul(
                        valsT[p][64:128, qcols], av[0:64, 512:1024],
                        bc_sb[0:64, 512:1024])
                return norm

            def attention_step(p, qb, fillers):
                qcols = slice(qb * 512, (qb + 1) * 512)
                ets = {}

                def qk(kc):
                    kc0 = slice(kc * 128, kc * 128 + 64)
                    kc1 = slice(kc * 128 + 64, kc * 128 + 128)
                    slot = qkps.tile([128, 1024], F32, name="qkslot")
                    # 4 concurrent 64x64-stationary matmuls, one per PE
                    # array quadrant (tile_position auto-derived from the
                    # lhsT/out base partitions).  Layout in PSUM matches
                    # the untiled version: cols 0:512 head 2p, 512:1024
                    # head 2p+1, partitions = key position within chunk.
                    nc.tensor.matmul(
                        slot[0:64, 0:512], kT[p][0:64, kc0],
                        qT[p][0:64, qcols], start=True, stop=True)
                    nc.tensor.matmul(
                        slot[0:64, 512:1024], kT[p][64:128, kc0],
                        qT[p][64:128, qcols], start=True, stop=True)
                    nc.tensor.matmul(
                        slot[64:128, 0:512], kT[p][0:64, kc1],
                        qT[p][0:64, qcols], start=True, stop=True)
                    nc.tensor.matmul(
                        slot[64:128, 512:1024], kT[p][64:128, kc1],
                        qT[p][64:128, qcols], start=True, stop=True)
                    et = eps.tile([128, 1024], BF16, name="et")
                    nc.scalar.activation(
                        et[:], slot[:], mybir.ActivationFunctionType.Exp)
                    ets[kc] = et

                qk(0)
                if pend_norm[0] is not None:
                    pend_norm[0]()
                av_t = avps.tile([128, 1024], F32, name="av")

                def av(kc):
                    et = ets.pop(kc)
                    nc.tensor.matmul(
                        av_t[0:65, 0:512], v_aug[:, kc, 2 * p, :],
                        et[:, 0:512],
                        start=(kc == 0), stop=(kc == NKC - 1))
                    nc.tensor.matmul(
                        av_t[0:65, 512:1024], v_aug[:, kc, 2 * p + 1, :],
                        et[:, 512:1024],
                        start=(kc == 0), stop=(kc == NKC - 1))

                qk(1)
                if fillers:
                    fillers[0]()
                for kc in range(2, NKC):
                    qk(kc)
                    av(kc - 2)
                if len(fillers) > 1:
                    fillers[1]()
                av(NKC - 2)
                av(NKC - 1)
                # Softmax denominators sit in av_t row 64 (ones column of
                # v_aug).  A direct DVE reciprocal on a [1, 1024] row is
                # ~9us (single lane); instead 32x32-block-transpose the
                # band rows 64:96 (row 64 = denominators; rows 65:96 are
                # stale psum that only lands in never-read lanes),
                # reciprocal just the strided lane holding the
                # denominators, and transpose back so the recip row lands
                # on partition 64 again.
                tband = rbs.tile([128, 1024], F32, name="tband")
                nc.vector.transpose(tband[64:96, :], av_t[64:96, 0:1024])
                tb3 = tband[64:96, :].rearrange("p (a b) -> p a b", b=32)
                f32_3 = rb_f32[64:96, :].rearrange("p (a b) -> p a b", b=32)
                b16_3 = rb_b16[64:96, :].rearrange("p (a b) -> p a b", b=32)
                nc.vector.reciprocal(
                    f32_3[:, :, 0:1], tb3[:, :, 0:1])
                nc.vector.tensor_copy(
                    b16_3[:, :, 0:1], f32_3[:, :, 0:1])
                recip = rbs.tile([128, 1024], BF16, name="recip")
                nc.vector.transpose(recip[64:96, :], rb_b16[64:96, :])
                pend_norm[0] = make_norm(p, qb, av_t, recip)

            # ---- prefix: v-proj (all kc) + q/k-proj for pair 0 ----
            for kc in range(NKC):
                vproj_group(kc)
            for qb in range(NQB):
                proj_group("q", 0, qb)
                proj_group("k", 0, qb)

            # ---- attention steps with interleaved proj / o-proj ----
            for i in range(NP * NQB):
                p, qb = i // NQB, i % NQB
                if i <= 11:
                    pn, j = i // 4 + 1, i % 4
                    fillers = [
                        (lambda pn=pn, j=j: proj_group("q", pn, j)),
                        (lambda pn=pn, j=j: proj_group("k", pn, j)),
                    ]
                elif i == 12:
                    fillers = []
                else:
                    blk = i - 13
                    fillers = [
                        (lambda blk=blk: (oproj_sc(4 * blk),
                                          oproj_sc(4 * blk + 1))),
                        (lambda blk=blk: (oproj_sc(4 * blk + 2),
                                          oproj_sc(4 * blk + 3))),
                    ]
                attention_step(p, qb, fillers)

            # ---- tail: last norm + last o-proj block ----
            pend_norm[0]()
            for sc in range(12, 16):
                oproj_sc(sc)

    nc.compile()
    return nc


def kernel(x, w_qkv, b_qkv, w_o, b_o):
    global LAST_EXEC_NS
    if _nc_cache[0] is None:
        _nc_cache[0] = _build()
    nc = _nc_cache[0]

    xT_b = [np.ascontiguousarray(x[b].T).astype(NBF).reshape(NIC, 128, S)
            for b in range(B)]
    w = w_qkv.astype(np.float32)
    in_maps = []
    for c in range(N_CORES):
        b, g = c // 2, c % 2
        # reference packs qkv per head: head h -> cols [h*192, (h+1)*192),
        # q dims 0:64, k 64:128, v 128:192 within
        heads = np.arange(g * 8, g * 8 + 8)
        qs = (heads[:, None] * 192 + np.arange(64)).ravel()
        ks = (heads[:, None] * 192 + 64 + np.arange(64)).ravel()
        vs = (heads[:, None] * 192 + 128 + np.arange(64)).ravel()
        in_maps.append({
            "xT": xT_b[b],
            "wq": (w[:, qs] / 8.0).astype(NBF).reshape(NIC, 128, 512),
            "wk": w[:, ks].astype(NBF).reshape(NIC, 128, 512),
            "wv": w[:, vs].astype(NBF).reshape(NIC, 128, 512),
            "wo": w_o[g * 512:(g + 1) * 512, :].astype(NBF).reshape(
                NP, 128, 1024),
            "bq": np.ascontiguousarray(
                (b_qkv[qs].astype(np.float32) / 8.0).reshape(NP, 128).T),
            "bk": np.ascontiguousarray(
                b_qkv[ks].astype(np.float32).reshape(NP, 128).T),
        })

    res = run_bass_kernel_spmd(nc, in_maps, list(range(N_CORES)),
                               trace=TRACE)
    LAST_EXEC_NS = res.exec_time_ns
    global LAST_RES
    LAST_RES = res

    # v-bias folds out of attention: softmax rows sum to 1, so
    # vals_h = p_h @ (x W_vh) + b_vh and the b_vh term contributes the
    # constant row (concat_h b_vh) @ w_o
    vs_full = (np.arange(H)[:, None] * 192 + 128 + np.arange(HD)).ravel()
    bvwo = b_qkv[vs_full].astype(np.float32) @ w_o.astype(np.float32)

    out = np.empty((B, S, E), np.float32)
    bias = b_o.astype(np.float32) + bvwo
    for b in range(B):
        p0 = np.asarray(res.results[2 * b]["out"],
                        np.float32).reshape(S, E)
        p1 = np.asarray(res.results[2 * b + 1]["out"],
                        np.float32).reshape(S, E)
        out[b] = p0 + p1 + bias
    return out

